# revision 1
# baseline (speedup 1.0000x reference)
"""Trainium2 Bass kernel for nn_LowpassDetector.

Computes power = re^2 + im^2 followed by a 4th-order Butterworth lowpass
IIR along the time axis (65536 steps, 512 channels).

Strategy: the IIR poles have max radius 0.7577, so the impulse response
decays below fp32 denormals within 128 taps (sum |h[j]| for j>=128 is
~7e-16).  A 256-tap FIR truncation is therefore numerically exact in
fp32.  The FIR is evaluated as two 128x128 Toeplitz matmuls per
128-timestep chunk (current chunk + previous chunk), which removes the
sequential dependence entirely:

    Y_chunk = H0 @ P_cur + H1 @ P_prev

This lets us shard TIME across the 8 cores (8192 steps each, with a
128-row input halo), giving fully contiguous DMA and zero communication.
Zero-padding the halo of core 0 reproduces the reference's zero initial
state exactly (for t < 256 the truncated FIR equals the IIR identically).
"""

import numpy as np

T_FULL = 65536
C = 512  # channels
NCORES = 8
TB = T_FULL // NCORES  # 8192 timesteps per core
CH = 128  # chunk length (matmul partition dim)
G = 4  # chunks per DMA group (1 MiB transfers)
GROUP_ROWS = G * CH  # 512
NG = TB // GROUP_ROWS  # 16 groups per core
HALO = CH
IN_ROWS = TB + HALO  # 8320
NTAPS = 2 * CH  # 256


def _impulse_response() -> np.ndarray:
    """256-tap impulse response of the reference Butterworth filter (float64)."""
    N, Wn = 4, 0.25
    m = np.arange(-N + 1, N, 2)
    p = -np.exp(1j * np.pi * m / (2 * N))
    fs = 2.0
    warped = 2.0 * fs * np.tan(np.pi * Wn / fs)
    p = p * warped
    k = warped**N
    fs2 = 2.0 * fs
    pz = (fs2 + p) / (fs2 - p)
    zz = -np.ones(N)
    kz = k * (1.0 / np.prod(fs2 - p)).real
    b = kz * np.real(np.poly(zz))
    a = np.real(np.poly(pz))
    b = b / a[0]
    a = a / a[0]
    z = np.zeros(N)
    h = np.zeros(NTAPS)
    for t in range(NTAPS):
        xt = 1.0 if t == 0 else 0.0
        y = b[0] * xt + z[0]
        z = np.concatenate([z[1:], [0.0]]) + b[1:] * xt - a[1:] * y
        h[t] = y
    return h


def _toeplitz_weights() -> tuple[np.ndarray, np.ndarray]:
    """lhsT weight matrices: Y = H0 @ P_cur + H1 @ P_prev, returned transposed."""
    h = _impulse_response()
    H0 = np.zeros((CH, CH))
    H1 = np.zeros((CH, CH))
    for i in range(CH):
        for ip in range(CH):
            if i - ip >= 0:
                H0[i, ip] = h[i - ip]
            H1[i, ip] = h[i - ip + CH]
    return (
        np.ascontiguousarray(H0.T).astype(np.float32),
        np.ascontiguousarray(H1.T).astype(np.float32),
    )


_BUILT = None


def _build():
    """Build + compile the Bass module (cached per process)."""
    global _BUILT
    if _BUILT is not None:
        return _BUILT

    import concourse.bacc as bacc
    import concourse.mybir as mybir
    import concourse.tile as tile

    f32 = mybir.dt.float32

    nc = bacc.Bacc(
        "TRN2",
        target_bir_lowering=False,
        debug=False,
        enable_asserts=False,
        num_devices=NCORES,
    )
    sig = nc.dram_tensor("sig", (2, IN_ROWS, C), f32, kind="ExternalInput").ap()
    h0t = nc.dram_tensor("h0t", (CH, CH), f32, kind="ExternalInput").ap()
    h1t = nc.dram_tensor("h1t", (CH, CH), f32, kind="ExternalInput").ap()
    y = nc.dram_tensor("y", (TB, C), f32, kind="ExternalOutput").ap()

    with tile.TileContext(nc) as tc:
        with (
            tc.tile_pool(name="consts", bufs=1) as cpool,
            tc.tile_pool(name="halo", bufs=1) as halo_pool,
            tc.tile_pool(name="re", bufs=4) as re_pool,
            tc.tile_pool(name="im", bufs=3) as im_pool,
            tc.tile_pool(name="out", bufs=3) as out_pool,
            tc.tile_pool(name="psum", bufs=6, space="PSUM") as psum_pool,
        ):
            h0_t = cpool.tile([CH, CH], f32, tag="h0")
            h1_t = cpool.tile([CH, CH], f32, tag="h1")
            nc.sync.dma_start(h0_t[:], h0t)
            nc.sync.dma_start(h1_t[:], h1t)

            # Halo chunk: power of input rows [0, 128) = timesteps [-128, 0)
            hre = halo_pool.tile([CH, C], f32, tag="hre")
            him = halo_pool.tile([CH, C], f32, tag="him")
            hp = halo_pool.tile([CH, C], f32, tag="hp")
            nc.sync.dma_start(hre[:], sig[0, 0:CH, :])
            nc.sync.dma_start(him[:], sig[1, 0:CH, :])
            nc.scalar.square(hre[:], hre[:])
            nc.scalar.square(him[:], him[:])
            nc.vector.tensor_add(hp[:], hre[:], him[:])

            prev_p = hp[:]  # power rows [-128, 0) of this core's range
            for g in range(NG):
                r0 = HALO + g * GROUP_ROWS  # input row offset
                re_t = re_pool.tile([CH, G, C], f32, tag="re")
                im_t = im_pool.tile([CH, G, C], f32, tag="im")
                nc.sync.dma_start(
                    re_t[:],
                    sig[0, r0 : r0 + GROUP_ROWS, :].rearrange(
                        "(g p) c -> p g c", p=CH
                    ),
                )
                nc.sync.dma_start(
                    im_t[:],
                    sig[1, r0 : r0 + GROUP_ROWS, :].rearrange(
                        "(g p) c -> p g c", p=CH
                    ),
                )
                # power in place: re_t <- re_t^2 + im_t^2
                nc.scalar.square(re_t[:], re_t[:])
                nc.scalar.square(im_t[:], im_t[:])
                nc.vector.tensor_add(re_t[:], re_t[:], im_t[:])

                out_t = out_pool.tile([CH, G, C], f32, tag="out")
                psums = []
                for j in range(G):
                    ps = psum_pool.tile([CH, C], f32, tag="ps")
                    nc.tensor.matmul(
                        ps[:], h0_t[:], re_t[:, j, :], start=True, stop=False
                    )
                    psums.append(ps)
                for j in range(G):
                    pv = prev_p if j == 0 else re_t[:, j - 1, :]
                    nc.tensor.matmul(
                        psums[j][:], h1_t[:], pv, start=False, stop=True
                    )
                for j in range(G):
                    nc.vector.tensor_copy(out_t[:, j, :], psums[j][:])

                nc.scalar.dma_start(
                    y[g * GROUP_ROWS : (g + 1) * GROUP_ROWS, :].rearrange(
                        "(g p) c -> p g c", p=CH
                    ),
                    out_t[:],
                )
                prev_p = re_t[:, G - 1, :]

    nc.compile()
    _BUILT = nc
    return nc


def _prepare_in_maps(signal: np.ndarray) -> list[dict[str, np.ndarray]]:
    h0t, h1t = _toeplitz_weights()
    signal = np.ascontiguousarray(np.asarray(signal, dtype=np.float32))
    assert signal.shape == (2, T_FULL, C), signal.shape
    in_maps = []
    for c in range(NCORES):
        t0 = c * TB
        if c == 0:
            block = np.concatenate(
                [np.zeros((2, HALO, C), np.float32), signal[:, 0:TB, :]], axis=1
            )
        else:
            block = signal[:, t0 - HALO : t0 + TB, :]
        in_maps.append(
            {
                "sig": np.ascontiguousarray(block),
                "h0t": h0t,
                "h1t": h1t,
            }
        )
    return in_maps


def _run(signal: np.ndarray, trace: bool = False):
    """Run the kernel; returns (full_output, BassKernelResults)."""
    from concourse import bass_utils

    nc = _build()
    in_maps = _prepare_in_maps(signal)
    results = bass_utils.run_bass_kernel_spmd(
        nc, in_maps, core_ids=list(range(NCORES)), trace=trace
    )
    y = np.concatenate([r["y"] for r in results.results], axis=0)
    return y, results


def kernel(signal: np.ndarray) -> np.ndarray:
    y, _ = _run(signal, trace=False)
    return y
